# revision 7
# baseline (speedup 1.0000x reference)
"""Llama4 MoE layer (top-1 routing) as an 8-core tensor-parallel Trainium2 kernel.

Sharding: tensor-parallel over the expert intermediate dim I. Every core holds
a distinct I/8 slice of ALL experts' gate/up (column-sharded) and down
(row-sharded) weights, and processes ALL tokens, grouped by routed expert so
the per-group matmuls use that expert's weight slice. Group boundaries are
global compile-time constants (identical on every core), so one SPMD program
serves all 8 cores; only the staged weight slices differ per core. Each core
produces a partial down-projection output (contraction over its I/8 slice);
the host sums the 8 partials and scatters tokens back to their original
positions (the "all-reduce on output" of the TP strategy, done host-side like
the baseline's host-side token dispatch). This removes expert-parallel
capacity padding entirely: per-core moving columns = T + small alignment pad
instead of 8*max(expert count).

All DRAM tensors are laid out so each DMA moves long contiguous per-partition
lines (8-36 KB): the HW-DGE path costs ~100ns/packet/engine, so KB-sized
packets (naive layouts) cap DMA at ~170 GB/s and starve the PE.

Device math per core, per expert e (n_e tokens, feature-on-partition layout):
    g[I/8,n] = gate_w_slice.T @ X.T, u likewise   (K = H, 16 k-tiles, 4 m-tiles)
    a[I/8,n] = silu(g) * u                        (bf16)
    y[H,n]  += down_w_slice.T @ a                 (K = I/8, 4 k-tiles, 16 m-tiles)
Matmuls in bf16, PSUM accumulation f32, partial outputs written as bf16.
"""

import numpy as np
import ml_dtypes

import concourse.bass as bass
import concourse.mybir as mybir
import concourse.tile as tile
from concourse import bacc
from concourse.bass_utils import run_bass_kernel_spmd

SEQ, BS, H, I, E = 2048, 4, 2048, 4096, 8
N_CORES = 8
P = 128
IS = I // N_CORES          # per-core I slice (512)
NKT = H // P               # 16 k-tiles over H (gate/up contraction; down m-tiles)
NMT_I = IS // P            # 4 m-tiles over I-slice (gate/up out; down contraction)
NCHUNK = 512               # matmul moving free dim / PSUM bank width (f32)
NSTRIPE = E * NMT_I        # 32 gate/up weight stripes

BF16 = mybir.dt.bfloat16
F32 = mybir.dt.float32
np_bf16 = ml_dtypes.bfloat16

# Stash of the last BassKernelResults (exec_time_ns when BASS_TRACE=1).
last_results = None


def _chunks(n):
    out = []
    off = 0
    while off < n:
        sz = min(NCHUNK, n - off)
        out.append((off, sz))
        off += sz
    return out


def _build(widths):
    """Build + compile the per-core TP program.

    widths: per-expert padded token-group column counts (in layout order);
    CT = sum(widths).
    """
    CT = sum(widths)
    n_max = max(widths)
    offs = np.concatenate([[0], np.cumsum(widths)]).astype(int)

    nc = bacc.Bacc("TRN2", target_bir_lowering=False, debug=False)
    # xt: per expert e, k-major flat [p, k*ne + j] at flat offset 16*off_e.
    xt_d = nc.dram_tensor("xt", [P, NKT * CT], BF16, kind="ExternalInput")
    # gate+up: per stripe s (= e*NMT_I + m), [p, (g/u)*NKT*P + k*P + i].
    guw_d = nc.dram_tensor("guw", [P, NSTRIPE, 2 * NKT * P], BF16, kind="ExternalInput")
    # down: per expert e, [p_i, m*NMT_I*P + k*P + p_h].
    dw_d = nc.dram_tensor("dw", [P, E, NKT * NMT_I * P], BF16, kind="ExternalInput")
    # out: per expert e, m-major flat [p, m*ne + j] at flat offset 16*off_e.
    out_d = nc.dram_tensor("out", [P, NKT * CT], BF16, kind="ExternalOutput")

    silu = mybir.ActivationFunctionType.Silu

    with tile.TileContext(nc) as tc:
        with (
            tc.tile_pool(name="xp", bufs=2) as xp,
            tc.tile_pool(name="wp", bufs=4) as wp,
            tc.tile_pool(name="dp", bufs=2) as dp,
            tc.tile_pool(name="ap", bufs=2) as ap,
            tc.tile_pool(name="sp", bufs=4) as sp,
            tc.tile_pool(name="op", bufs=2) as op,
            tc.tile_pool(name="pp", bufs=8, space="PSUM") as pp,
        ):
            # Warm the PE HAM clock gate with dummy matmuls while the first
            # DMAs are in flight (no data deps — zeros in, discarded out).
            warm = sp.tile([P, NCHUNK], BF16, tag="warm", bufs=1)
            nc.gpsimd.memset(warm[:], 0.0)
            warm_ps = pp.tile([P, NCHUNK], F32, tag="ps")
            for _ in range(16):
                nc.tensor.matmul(
                    warm_ps[:], warm[:, :P], warm[:], start=True, stop=True
                )

            GUW = 2 * NKT * P  # flat stripe length (gate half then up half)

            # Expert 0 prologue: gate half of stripe 0 first, then xt group 0
            # in quarters, so the first matmuls' inputs land within ~1-2µs.
            n0 = widths[0]
            xt_t0 = xp.tile([P, NKT * n_max], BF16, tag="xt")
            guw_t0 = wp.tile([P, GUW], BF16, tag="guw")
            guw_t1 = wp.tile([P, GUW], BF16, tag="guw")
            nc.sync.dma_start(guw_t0[:, : NKT * P], guw_d[:, 0, : NKT * P])
            for q in range(4):
                a, b = 4 * q * n0, 4 * (q + 1) * n0
                nc.sync.dma_start(xt_t0[:, a:b], xt_d[:, a:b])
            nc.sync.dma_start(guw_t0[:, NKT * P :], guw_d[:, 0, NKT * P :])
            nc.sync.dma_start(guw_t1[:], guw_d[:, 1])

            for e in range(E):
                ne = widths[e]
                chunks = _chunks(ne)
                off = int(offs[e])
                fbase = NKT * off  # flat base of this expert's xt/out block

                if e == 0:
                    xt_t = xt_t0
                else:
                    xt_t = xp.tile([P, NKT * n_max], BF16, tag="xt")
                    for q in range(4):
                        a, b = 4 * q * ne, 4 * (q + 1) * ne
                        nc.sync.dma_start(
                            xt_t[:, a:b], xt_d[:, fbase + a : fbase + b]
                        )

                act = ap.tile([P, NMT_I, n_max], BF16, tag="act")

                # ---- gate/up + silu*up for this expert's token group ----
                for m in range(NMT_I):
                    s = e * NMT_I + m
                    if s == 0:
                        guw_t = guw_t0
                    elif s == 1:
                        guw_t = guw_t1
                    else:
                        guw_t = wp.tile([P, GUW], BF16, tag="guw")
                        nc.sync.dma_start(guw_t[:], guw_d[:, s])

                    psg = [pp.tile([P, sz], F32, tag="ps", name=f"psg{e}_{m}_{ci}")
                           for ci, (_, sz) in enumerate(chunks)]
                    for k in range(NKT):
                        for ci, (o, sz) in enumerate(chunks):
                            nc.tensor.matmul(
                                psg[ci][:],
                                guw_t[:, k * P : (k + 1) * P],
                                xt_t[:, k * ne + o : k * ne + o + sz],
                                start=(k == 0),
                                stop=(k == NKT - 1),
                            )
                    psu = [pp.tile([P, sz], F32, tag="ps", name=f"psu{e}_{m}_{ci}")
                           for ci, (_, sz) in enumerate(chunks)]
                    for k in range(NKT):
                        for ci, (o, sz) in enumerate(chunks):
                            nc.tensor.matmul(
                                psu[ci][:],
                                guw_t[:, (NKT + k) * P : (NKT + k + 1) * P],
                                xt_t[:, k * ne + o : k * ne + o + sz],
                                start=(k == 0),
                                stop=(k == NKT - 1),
                            )
                    for ci, (o, sz) in enumerate(chunks):
                        sil = sp.tile([P, NCHUNK], F32, tag="sil")
                        nc.scalar.activation(sil[:, :sz], psg[ci][:], silu)
                        nc.vector.tensor_mul(
                            act[:, m, o : o + sz], sil[:, :sz], psu[ci][:]
                        )

                # ---- down (partial over this core's I slice) ----
                dw_t = dp.tile([P, NKT * NMT_I * P], BF16, tag="dw")
                nc.sync.dma_start(dw_t[:], dw_d[:, e])
                last_e = e == E - 1
                for half in range(2):
                    ot = op.tile([P, 8 * n_max], BF16, tag="ot")
                    for mm in range(8):
                        m = half * 8 + mm
                        psd = [pp.tile([P, sz], F32, tag="ps", name=f"psd{e}_{m}_{ci}")
                               for ci, (_, sz) in enumerate(chunks)]
                        last_m = last_e and m == NKT - 1
                        if not last_m:
                            for k in range(NMT_I):
                                for ci, (o, sz) in enumerate(chunks):
                                    nc.tensor.matmul(
                                        psd[ci][:],
                                        dw_t[:, (m * NMT_I + k) * P : (m * NMT_I + k + 1) * P],
                                        act[:, k, o : o + sz],
                                        start=(k == 0),
                                        stop=(k == NMT_I - 1),
                                    )
                            for ci, (o, sz) in enumerate(chunks):
                                if ci % 2 == 0:
                                    nc.vector.tensor_copy(
                                        ot[:, mm * ne + o : mm * ne + o + sz],
                                        psd[ci][:],
                                    )
                                else:
                                    nc.scalar.copy(
                                        ot[:, mm * ne + o : mm * ne + o + sz],
                                        psd[ci][:],
                                    )
                            if last_e and half == 1:
                                # fine-grained drain (alternating DMA queues)
                                # so the tail doesn't wait on one queue's
                                # serialized issue + backlog.
                                nc.sync.dma_start(
                                    out_d[:, fbase + m * ne : fbase + (m + 1) * ne],
                                    ot[:, mm * ne : (mm + 1) * ne],
                                )
                        else:
                            # very last m-tile: finish chunks one at a time so
                            # their output DMAs drain before the exit barrier.
                            for ci, (o, sz) in enumerate(chunks):
                                for k in range(NMT_I):
                                    nc.tensor.matmul(
                                        psd[ci][:],
                                        dw_t[:, (m * NMT_I + k) * P : (m * NMT_I + k + 1) * P],
                                        act[:, k, o : o + sz],
                                        start=(k == 0),
                                        stop=(k == NMT_I - 1),
                                    )
                                ot2 = sp.tile([P, NCHUNK], BF16, tag="otail")
                                nc.vector.tensor_copy(ot2[:, :sz], psd[ci][:])
                                nc.sync.dma_start(
                                    out_d[:, fbase + m * ne + o : fbase + m * ne + o + sz],
                                    ot2[:, :sz],
                                )
                    if not (last_e and half == 1):
                        nc.sync.dma_start(
                            out_d[:, fbase + half * 8 * ne : fbase + (half + 1) * 8 * ne],
                            ot[:, : 8 * ne],
                        )

    nc.compile()
    return nc


def kernel(hidden_states, router_w, gate_w, up_w, down_w):
    global last_results
    X = np.asarray(hidden_states, dtype=np.float32).reshape(-1, H)
    router_w = np.asarray(router_w, dtype=np.float32)
    gate_w = np.asarray(gate_w, dtype=np.float32)
    up_w = np.asarray(up_w, dtype=np.float32)
    down_w = np.asarray(down_w, dtype=np.float32)
    T = X.shape[0]

    # --- routing (sharding layer): top-1 expert per token ---
    # Mirror the reference's routing computation op-for-op (jnp.einsum +
    # argmax) so near-tied logits resolve to the same expert.
    import jax.numpy as jnp

    logits = jnp.einsum(
        "sbh,he->sbe", np.asarray(hidden_states, dtype=np.float32), router_w
    )
    eid = np.asarray(jnp.argmax(logits, axis=-1)).reshape(-1)  # [T]
    idx = [np.nonzero(eid == e)[0] for e in range(E)]

    # Column layout: token groups ordered small-first (fast startup) with the
    # smallest group LAST (small tail drain); padded to multiples of 4
    # (8-byte-aligned bf16 rows).
    asc = sorted(range(E), key=lambda e: len(idx[e]))
    order = asc[1:] + asc[:1]
    widths = [max(4, ((len(idx[e]) + 3) // 4) * 4) for e in order]
    CT = sum(widths)
    offs = np.concatenate([[0], np.cumsum(widths)]).astype(int)

    # xt: per expert, [p, k, j] = X[token j of group, k*128+p], flattened.
    xt = np.zeros((P, NKT * CT), np_bf16)
    for gi, e in enumerate(order):
        ix = idx[e]
        ne = widths[gi]
        blk = np.zeros((ne, H), np.float32)
        blk[: len(ix)] = X[ix]
        # [H, ne] -> [k,p,ne] -> [p,k,ne] -> [p, k*ne]
        arr = blk.T.reshape(NKT, P, ne).transpose(1, 0, 2).reshape(P, NKT * ne)
        xt[:, NKT * offs[gi] : NKT * offs[gi + 1]] = arr.astype(np_bf16)

    in_maps = []
    for c in range(N_CORES):
        sl = slice(c * IS, (c + 1) * IS)
        # gate/up [E,H,IS] -> [p, e*4+m, k, i] -> [P, 32, NKT*P] per half
        gs = np.stack([gate_w[e][:, sl] for e in order])  # [E, H, IS]
        us = np.stack([up_w[e][:, sl] for e in order])
        ga = (
            gs.reshape(E, NKT, P, NMT_I, P)
            .transpose(2, 0, 3, 1, 4)  # [p, e, m, k, i]
            .reshape(P, NSTRIPE, NKT * P)
        )
        ua = (
            us.reshape(E, NKT, P, NMT_I, P)
            .transpose(2, 0, 3, 1, 4)
            .reshape(P, NSTRIPE, NKT * P)
        )
        guw = np.concatenate([ga, ua], axis=2).astype(np_bf16)  # [P, 32, 2*NKT*P]
        # down [E,IS,H] -> [p_i, e, m, k, p_h] -> [P, E, NKT*NMT_I*P]
        ds = np.stack([down_w[e][sl, :] for e in order])  # [E, IS, H]
        dw = (
            ds.reshape(E, NMT_I, P, NKT, P)
            .transpose(2, 0, 3, 1, 4)  # [p_i, e, m, k, p_h]
            .reshape(P, E, NKT * NMT_I * P)
            .astype(np_bf16)
        )
        in_maps.append({"xt": xt, "guw": guw, "dw": dw})

    nc = _build(widths)
    last_results = run_bass_kernel_spmd(nc, in_maps, list(range(N_CORES)))

    # Host-side "all-reduce": sum the 8 bf16 partials in f32, then scatter.
    acc = np.zeros((P, NKT * CT), np.float32)
    for c in range(N_CORES):
        acc += np.asarray(last_results.results[c]["out"]).astype(np.float32)

    out = np.zeros((T, H), np.float32)
    for gi, e in enumerate(order):
        ix = idx[e]
        ne = widths[gi]
        blk = acc[:, NKT * offs[gi] : NKT * offs[gi + 1]].reshape(P, NKT, ne)
        full = blk.transpose(1, 0, 2).reshape(H, ne)  # rows h = m*128 + p
        out[ix] = full[:, : len(ix)].T
    return out.reshape(SEQ, BS, H)


# revision 8
# speedup vs baseline: 1.0042x; 1.0042x over previous
"""Llama4 MoE layer (top-1 routing) as an 8-core tensor-parallel Trainium2 kernel.

Sharding: tensor-parallel over the expert intermediate dim I. Every core holds
a distinct I/8 slice of ALL experts' gate/up (column-sharded) and down
(row-sharded) weights, and processes ALL tokens, grouped by routed expert so
the per-group matmuls use that expert's weight slice. Group boundaries are
global compile-time constants (identical on every core), so one SPMD program
serves all 8 cores; only the staged weight slices differ per core. Each core
produces a partial down-projection output (contraction over its I/8 slice);
the host sums the 8 partials and scatters tokens back to their original
positions (the "all-reduce on output" of the TP strategy, done host-side like
the baseline's host-side token dispatch). This removes expert-parallel
capacity padding entirely: per-core moving columns = T + small alignment pad
instead of 8*max(expert count).

All DRAM tensors are laid out so each DMA moves long contiguous per-partition
lines (8-36 KB): the HW-DGE path costs ~100ns/packet/engine, so KB-sized
packets (naive layouts) cap DMA at ~170 GB/s and starve the PE.

Device math per core, per expert e (n_e tokens, feature-on-partition layout):
    g[I/8,n] = gate_w_slice.T @ X.T, u likewise   (K = H, 16 k-tiles, 4 m-tiles)
    a[I/8,n] = silu(g) * u                        (bf16)
    y[H,n]  += down_w_slice.T @ a                 (K = I/8, 4 k-tiles, 16 m-tiles)
Matmuls in bf16, PSUM accumulation f32, partial outputs written as bf16.
"""

import numpy as np
import ml_dtypes

import concourse.bass as bass
import concourse.mybir as mybir
import concourse.tile as tile
from concourse import bacc
from concourse.bass_utils import run_bass_kernel_spmd

SEQ, BS, H, I, E = 2048, 4, 2048, 4096, 8
N_CORES = 8
P = 128
IS = I // N_CORES          # per-core I slice (512)
NKT = H // P               # 16 k-tiles over H (gate/up contraction; down m-tiles)
NMT_I = IS // P            # 4 m-tiles over I-slice (gate/up out; down contraction)
NCHUNK = 512               # matmul moving free dim / PSUM bank width (f32)
NSTRIPE = E * NMT_I        # 32 gate/up weight stripes

BF16 = mybir.dt.bfloat16
F32 = mybir.dt.float32
np_bf16 = ml_dtypes.bfloat16

# Stash of the last BassKernelResults (exec_time_ns when BASS_TRACE=1).
last_results = None


def _chunks(n):
    out = []
    off = 0
    while off < n:
        sz = min(NCHUNK, n - off)
        out.append((off, sz))
        off += sz
    return out


def _build(widths):
    """Build + compile the per-core TP program.

    widths: per-expert padded token-group column counts (in layout order);
    CT = sum(widths).
    """
    CT = sum(widths)
    n_max = max(widths)
    offs = np.concatenate([[0], np.cumsum(widths)]).astype(int)

    nc = bacc.Bacc("TRN2", target_bir_lowering=False, debug=False)
    # xt: per expert e, k-major flat [p, k*ne + j] at flat offset 16*off_e.
    xt_d = nc.dram_tensor("xt", [P, NKT * CT], BF16, kind="ExternalInput")
    # gate+up: per stripe s (= e*NMT_I + m), [p, (g/u)*NKT*P + k*P + i].
    guw_d = nc.dram_tensor("guw", [P, NSTRIPE, 2 * NKT * P], BF16, kind="ExternalInput")
    # down: per expert e, [p_i, m*NMT_I*P + k*P + p_h].
    dw_d = nc.dram_tensor("dw", [P, E, NKT * NMT_I * P], BF16, kind="ExternalInput")
    # out: per expert e, m-major flat [p, m*ne + j] at flat offset 16*off_e.
    out_d = nc.dram_tensor("out", [P, NKT * CT], BF16, kind="ExternalOutput")

    silu = mybir.ActivationFunctionType.Silu

    with tile.TileContext(nc) as tc:
        with (
            tc.tile_pool(name="xp", bufs=2) as xp,
            tc.tile_pool(name="wp", bufs=3) as wp,
            tc.tile_pool(name="dp", bufs=2) as dp,
            tc.tile_pool(name="ap", bufs=2) as ap,
            tc.tile_pool(name="sp", bufs=4) as sp,
            tc.tile_pool(name="op", bufs=2) as op,
            tc.tile_pool(name="pp", bufs=8, space="PSUM") as pp,
        ):
            # Warm the PE HAM clock gate with dummy matmuls while the first
            # DMAs are in flight (no data deps — zeros in, discarded out).
            warm = sp.tile([P, NCHUNK], BF16, tag="warm", bufs=1)
            nc.gpsimd.memset(warm[:], 0.0)
            warm_ps = pp.tile([P, NCHUNK], F32, tag="ps")
            for _ in range(12):
                nc.tensor.matmul(
                    warm_ps[:], warm[:, :P], warm[:], start=True, stop=True
                )

            GUW = 2 * NKT * P  # flat stripe length (gate half then up half)

            # Expert 0 prologue: gate half of stripe 0 first, then xt group 0
            # in quarters, so the first matmuls' inputs land within ~1-2µs.
            n0 = widths[0]
            xt_t0 = xp.tile([P, NKT * n_max], BF16, tag="xt")
            guw_t0 = wp.tile([P, GUW], BF16, tag="guw")
            guw_t1 = wp.tile([P, GUW], BF16, tag="guw")
            nc.sync.dma_start(guw_t0[:, : NKT * P], guw_d[:, 0, : NKT * P])
            for q in range(4):
                a, b = 4 * q * n0, 4 * (q + 1) * n0
                nc.sync.dma_start(xt_t0[:, a:b], xt_d[:, a:b])
            nc.sync.dma_start(guw_t0[:, NKT * P :], guw_d[:, 0, NKT * P :])
            nc.sync.dma_start(guw_t1[:], guw_d[:, 1])

            for e in range(E):
                ne = widths[e]
                chunks = _chunks(ne)
                off = int(offs[e])
                fbase = NKT * off  # flat base of this expert's xt/out block

                if e == 0:
                    xt_t = xt_t0
                else:
                    xt_t = xp.tile([P, NKT * n_max], BF16, tag="xt")
                    for q in range(4):
                        a, b = 4 * q * ne, 4 * (q + 1) * ne
                        nc.sync.dma_start(
                            xt_t[:, a:b], xt_d[:, fbase + a : fbase + b]
                        )

                act = ap.tile([P, NMT_I, n_max], BF16, tag="act")

                # ---- gate/up + silu*up for this expert's token group ----
                for m in range(NMT_I):
                    s = e * NMT_I + m
                    if s == 0:
                        guw_t = guw_t0
                    elif s == 1:
                        guw_t = guw_t1
                    else:
                        guw_t = wp.tile([P, GUW], BF16, tag="guw")
                        nc.sync.dma_start(guw_t[:], guw_d[:, s])

                    psg = [pp.tile([P, sz], F32, tag="ps", name=f"psg{e}_{m}_{ci}")
                           for ci, (_, sz) in enumerate(chunks)]
                    for k in range(NKT):
                        for ci, (o, sz) in enumerate(chunks):
                            nc.tensor.matmul(
                                psg[ci][:],
                                guw_t[:, k * P : (k + 1) * P],
                                xt_t[:, k * ne + o : k * ne + o + sz],
                                start=(k == 0),
                                stop=(k == NKT - 1),
                            )
                    psu = [pp.tile([P, sz], F32, tag="ps", name=f"psu{e}_{m}_{ci}")
                           for ci, (_, sz) in enumerate(chunks)]
                    for k in range(NKT):
                        for ci, (o, sz) in enumerate(chunks):
                            nc.tensor.matmul(
                                psu[ci][:],
                                guw_t[:, (NKT + k) * P : (NKT + k + 1) * P],
                                xt_t[:, k * ne + o : k * ne + o + sz],
                                start=(k == 0),
                                stop=(k == NKT - 1),
                            )
                    for ci, (o, sz) in enumerate(chunks):
                        sil = sp.tile([P, NCHUNK], F32, tag="sil")
                        nc.scalar.activation(sil[:, :sz], psg[ci][:], silu)
                        nc.vector.tensor_mul(
                            act[:, m, o : o + sz], sil[:, :sz], psu[ci][:]
                        )

                # ---- down (partial over this core's I slice) ----
                dw_t = dp.tile([P, NKT * NMT_I * P], BF16, tag="dw")
                nc.sync.dma_start(dw_t[:], dw_d[:, e])
                last_e = e == E - 1
                for half in range(2):
                    ot = op.tile([P, 8 * n_max], BF16, tag="ot")
                    for mm in range(8):
                        m = half * 8 + mm
                        psd = [pp.tile([P, sz], F32, tag="ps", name=f"psd{e}_{m}_{ci}")
                               for ci, (_, sz) in enumerate(chunks)]
                        last_m = last_e and m == NKT - 1
                        if not last_m:
                            for k in range(NMT_I):
                                for ci, (o, sz) in enumerate(chunks):
                                    nc.tensor.matmul(
                                        psd[ci][:],
                                        dw_t[:, (m * NMT_I + k) * P : (m * NMT_I + k + 1) * P],
                                        act[:, k, o : o + sz],
                                        start=(k == 0),
                                        stop=(k == NMT_I - 1),
                                    )
                            for ci, (o, sz) in enumerate(chunks):
                                if ci % 2 == 0:
                                    nc.vector.tensor_copy(
                                        ot[:, mm * ne + o : mm * ne + o + sz],
                                        psd[ci][:],
                                    )
                                else:
                                    nc.scalar.copy(
                                        ot[:, mm * ne + o : mm * ne + o + sz],
                                        psd[ci][:],
                                    )
                            if last_e and half == 1:
                                # fine-grained drain (alternating DMA queues)
                                # so the tail doesn't wait on one queue's
                                # serialized issue + backlog.
                                nc.sync.dma_start(
                                    out_d[:, fbase + m * ne : fbase + (m + 1) * ne],
                                    ot[:, mm * ne : (mm + 1) * ne],
                                )
                        else:
                            # very last m-tile: finish chunks one at a time so
                            # their output DMAs drain before the exit barrier.
                            for ci, (o, sz) in enumerate(chunks):
                                for k in range(NMT_I):
                                    nc.tensor.matmul(
                                        psd[ci][:],
                                        dw_t[:, (m * NMT_I + k) * P : (m * NMT_I + k + 1) * P],
                                        act[:, k, o : o + sz],
                                        start=(k == 0),
                                        stop=(k == NMT_I - 1),
                                    )
                                ot2 = sp.tile([P, NCHUNK], BF16, tag="otail")
                                nc.vector.tensor_copy(ot2[:, :sz], psd[ci][:])
                                nc.sync.dma_start(
                                    out_d[:, fbase + m * ne + o : fbase + m * ne + o + sz],
                                    ot2[:, :sz],
                                )
                    if not (last_e and half == 1):
                        nc.sync.dma_start(
                            out_d[:, fbase + half * 8 * ne : fbase + (half + 1) * 8 * ne],
                            ot[:, : 8 * ne],
                        )

    nc.compile()
    return nc


def kernel(hidden_states, router_w, gate_w, up_w, down_w):
    global last_results
    X = np.asarray(hidden_states, dtype=np.float32).reshape(-1, H)
    router_w = np.asarray(router_w, dtype=np.float32)
    gate_w = np.asarray(gate_w, dtype=np.float32)
    up_w = np.asarray(up_w, dtype=np.float32)
    down_w = np.asarray(down_w, dtype=np.float32)
    T = X.shape[0]

    # --- routing (sharding layer): top-1 expert per token ---
    # Mirror the reference's routing computation op-for-op (jnp.einsum +
    # argmax) so near-tied logits resolve to the same expert.
    import jax.numpy as jnp

    logits = jnp.einsum(
        "sbh,he->sbe", np.asarray(hidden_states, dtype=np.float32), router_w
    )
    eid = np.asarray(jnp.argmax(logits, axis=-1)).reshape(-1)  # [T]
    idx = [np.nonzero(eid == e)[0] for e in range(E)]

    # Column layout: token groups ordered small-first (fast startup) with the
    # smallest group LAST (small tail drain); padded to multiples of 4
    # (8-byte-aligned bf16 rows).
    asc = sorted(range(E), key=lambda e: len(idx[e]))
    order = asc[1:] + asc[:1]
    widths = [max(4, ((len(idx[e]) + 3) // 4) * 4) for e in order]
    CT = sum(widths)
    offs = np.concatenate([[0], np.cumsum(widths)]).astype(int)

    # xt: per expert, [p, k, j] = X[token j of group, k*128+p], flattened.
    xt = np.zeros((P, NKT * CT), np_bf16)
    for gi, e in enumerate(order):
        ix = idx[e]
        ne = widths[gi]
        blk = np.zeros((ne, H), np.float32)
        blk[: len(ix)] = X[ix]
        # [H, ne] -> [k,p,ne] -> [p,k,ne] -> [p, k*ne]
        arr = blk.T.reshape(NKT, P, ne).transpose(1, 0, 2).reshape(P, NKT * ne)
        xt[:, NKT * offs[gi] : NKT * offs[gi + 1]] = arr.astype(np_bf16)

    in_maps = []
    for c in range(N_CORES):
        sl = slice(c * IS, (c + 1) * IS)
        # gate/up [E,H,IS] -> [p, e*4+m, k, i] -> [P, 32, NKT*P] per half
        gs = np.stack([gate_w[e][:, sl] for e in order])  # [E, H, IS]
        us = np.stack([up_w[e][:, sl] for e in order])
        ga = (
            gs.reshape(E, NKT, P, NMT_I, P)
            .transpose(2, 0, 3, 1, 4)  # [p, e, m, k, i]
            .reshape(P, NSTRIPE, NKT * P)
        )
        ua = (
            us.reshape(E, NKT, P, NMT_I, P)
            .transpose(2, 0, 3, 1, 4)
            .reshape(P, NSTRIPE, NKT * P)
        )
        guw = np.concatenate([ga, ua], axis=2).astype(np_bf16)  # [P, 32, 2*NKT*P]
        # down [E,IS,H] -> [p_i, e, m, k, p_h] -> [P, E, NKT*NMT_I*P]
        ds = np.stack([down_w[e][sl, :] for e in order])  # [E, IS, H]
        dw = (
            ds.reshape(E, NMT_I, P, NKT, P)
            .transpose(2, 0, 3, 1, 4)  # [p_i, e, m, k, p_h]
            .reshape(P, E, NKT * NMT_I * P)
            .astype(np_bf16)
        )
        in_maps.append({"xt": xt, "guw": guw, "dw": dw})

    nc = _build(widths)
    last_results = run_bass_kernel_spmd(nc, in_maps, list(range(N_CORES)))

    # Host-side "all-reduce": sum the 8 bf16 partials in f32, then scatter.
    acc = np.zeros((P, NKT * CT), np.float32)
    for c in range(N_CORES):
        acc += np.asarray(last_results.results[c]["out"]).astype(np.float32)

    out = np.zeros((T, H), np.float32)
    for gi, e in enumerate(order):
        ix = idx[e]
        ne = widths[gi]
        blk = acc[:, NKT * offs[gi] : NKT * offs[gi + 1]].reshape(P, NKT, ne)
        full = blk.transpose(1, 0, 2).reshape(H, ne)  # rows h = m*128 + p
        out[ix] = full[:, : len(ix)].T
    return out.reshape(SEQ, BS, H)
